# revision 54
# baseline (speedup 1.0000x reference)
"""DegreeGCNLayer on 8 Trainium2 NeuronCores (Bass/Tile, SPMD).

h = (segment_sum(feature[src] * rsqrt(deg)[src], dst) * rsqrt(deg)) @ W + b

Sharding: nodes split 8 ways (9375/core); edges partitioned by dst owner on
the host so the segment-sum is core-local; feature/degree replicated to every
core (host-side replication stands in for the all-gather of remote src
features); W/b replicated.

Per-core device program:
  1. pre-scale  f~ = feature * rsqrt(degree)  -> DRAM scratch (3 chunk tables,
     32768 rows each so gather indices fit int16)
  2. dma_gather f~[src_e] per edge (SWDGE MoE gather, 1024-edge calls; edges
     sorted by (src-chunk, dst-window) on host)
  3. segment-sum on the PE: per 128-edge group build a 0/1 selection matrix
     S[j, r] = (dst_rel[j] == r) on DVE (iota compare), then
     psum_window[128 rows, 64] += S.T @ messages — exact, deterministic.
     (dma_scatter_add loses concurrent duplicate-row adds on HW, so the DMA
     CCE-add path is unusable for segment_sum.)
  4. window flush: psum -> SBUF-resident agg table (DVE add)
  5. finalize per window: scale rows by rsqrt(deg_own), PE-transpose,
     PE matmul against [W; b] (bias folded via a ones row), DMA out.
"""

import numpy as np

from concourse import bacc, bass, mybir, tile
from concourse.bass_utils import run_bass_kernel_spmd
from concourse.masks import make_identity

N_NODES = 75000
N_EDGES = 1200000
F = 64
NCORES = 8
OWN = N_NODES // NCORES            # 9375
CHUNK = 32768                      # int16-indexable gather table chunk
PAD_N = 75008                      # 9 tiles of 8192 + 1 tile of 1280
CHUNK_ROWS = (CHUNK, CHUNK, PAD_N - 2 * CHUNK)   # 32768, 32768, 9472
TILE_E = 1024                      # edges per gather call (SWDGE ring holds
                                   # 128 in-flight entries; >=2048 idxs per
                                   # call deadlocks on HW)
CHUNK_ORDER = (2, 0, 1)            # chunk 2 has the smallest pre-scale (2
                                   # tiles), so starting with it minimizes the
                                   # pipeline head before the first gather
AGG_ROWS = 9472                    # 74 windows of 128 rows (>= OWN)
N_WIN = AGG_ROWS // 128            # 74
F32 = mybir.dt.float32
I16 = mybir.dt.int16
I32 = mybir.dt.int32


def _build_nc(structure, caps, gbufs=6, sbufs=4, psbufs=4, nqueues=1,
              scratch=16384,
              ab_prescale=True, ab_gather=True, ab_sbuild=True,
              ab_matmul=True, ab_final=True):
    """Build the single SPMD Bass program.

    structure: per chunk, list of (window, n_groups) in stream order.
    caps: per-chunk padded token counts (multiples of TILE_E).
    """
    nc = bacc.Bacc("TRN2", target_bir_lowering=False, debug=False,
                   num_swdge_queues=nqueues,
                   dynamic_dma_scratch_size=scratch)

    feat = nc.declare_dram_parameter("feature", [PAD_N, F], F32, isOutput=False)
    deg = nc.declare_dram_parameter("degree", [PAD_N], F32, isOutput=False)
    tot = sum(caps)
    gidx = nc.declare_dram_parameter("gidx", [128, tot // 16], I16, isOutput=False)
    drel = nc.declare_dram_parameter("drel", [128, tot // 128], I16, isOutput=False)
    deg_own = nc.declare_dram_parameter("deg_own", [AGG_ROWS], F32, isOutput=False)
    w_in = nc.declare_dram_parameter("W", [F, F], F32, isOutput=False)
    b_in = nc.declare_dram_parameter("b", [F], F32, isOutput=False)
    out = nc.declare_dram_parameter("out", [OWN, F], F32, isOutput=True)

    ftabs = [nc.dram_tensor(f"ftab{c}", [CHUNK_ROWS[c], F], F32) for c in range(3)]

    with tile.TileContext(nc) as tc:
        with (
            tc.tile_pool(name="const", bufs=1) as constp,
            tc.tile_pool(name="idxp", bufs=1) as idxp,
            tc.tile_pool(name="aggp", bufs=1) as aggp,
        ):
            # --- resident constants -------------------------------------
            gidx_sb = idxp.tile([128, tot // 16], I16, tag="gidx")
            drel_sb = idxp.tile([128, tot // 128], I16, tag="drel")
            nc.sync.dma_start(out=gidx_sb[:, :], in_=gidx[:, :])
            nc.sync.dma_start(out=drel_sb[:, :], in_=drel[:, :])

            wb = constp.tile([F, F], F32, tag="wb")
            nc.sync.dma_start(out=wb[:, :], in_=w_in[:, :])

            ident = constp.tile([128, 128], F32, tag="ident")
            make_identity(nc, ident[:, :])
            ident74 = constp.tile([N_WIN, N_WIN], F32, tag="ident74")
            make_identity(nc, ident74[:, :])

            iota_mat = constp.tile([128, 128], I16, tag="iota_mat")
            nc.gpsimd.iota(iota_mat[:, :], pattern=[[1, 128]], base=0,
                           channel_multiplier=0)

            # b broadcast to all partitions via K=1 outer product with ones
            ones_row = constp.tile([1, 128], F32, tag="ones_row")
            nc.vector.memset(ones_row[:, :], 1.0)
            b_row = constp.tile([1, F], F32, tag="b_row")
            nc.sync.dma_start(out=b_row[:, :], in_=b_in[:].unsqueeze(0))
            with tc.tile_pool(name="psb", bufs=1, space="PSUM") as psbp:
                bm_ps = psbp.tile([128, F], F32, tag="bm_ps")
                nc.tensor.matmul(
                    out=bm_ps[:, :], lhsT=ones_row[:, :], rhs=b_row[:, :],
                    start=True, stop=True,
                )
                b_mat = constp.tile([128, F], F32, tag="b_mat")
                nc.vector.tensor_copy(b_mat[:, :], bm_ps[:, :])

            # rsqrt(deg_own): [74,128] recip+sqrt, PE-transpose -> [128,74]
            so_raw = constp.tile([N_WIN, 128], F32, tag="so_raw")
            nc.sync.dma_start(
                out=so_raw[:, :],
                in_=deg_own[:].rearrange("(w p) -> w p", w=N_WIN),
            )
            nc.vector.reciprocal(so_raw[:, :], so_raw[:, :])
            nc.scalar.sqrt(so_raw[:, :], so_raw[:, :])
            with tc.tile_pool(name="psc", bufs=1, space="PSUM") as pscp:
                so_ps = pscp.tile([128, N_WIN], F32, tag="so_ps")
                nc.tensor.transpose(
                    out=so_ps[:, :], in_=so_raw[:, :], identity=ident74[:, :]
                )
                s_own = constp.tile([128, N_WIN], F32, tag="s_own")
                nc.vector.tensor_copy(s_own[:, :], so_ps[:, :])

            # --- SBUF-resident agg accumulator + output buffer ----------
            agg_sb = aggp.tile([128, N_WIN, F], F32, tag="agg")
            nc.vector.memset(agg_sb[:, :, :], 0.0)
            osb_all = aggp.tile([128, N_WIN, F], F32, tag="osb_all")

            # --- per-chunk pre-scale tiles (emitted just-in-time) -------
            chunk_tiles = [
                [(t * 8192, 64) for t in range(4)],
                [(t * 8192, 64) for t in range(4, 8)],
                [(8 * 8192, 64), (73728, 10)],
            ]

            def prescale(pp, c):
                if not ab_prescale:
                    return
                for row0, G in chunk_tiles[c]:
                    nrow = 128 * G
                    ft = pp.tile([128, 64, F], F32, tag="ft")
                    dg = pp.tile([128, 64], F32, tag="dg")
                    nc.sync.dma_start(
                        out=ft[:, :G, :],
                        in_=feat[row0 : row0 + nrow, :].rearrange(
                            "(p g) f -> p g f", p=128
                        ),
                    )
                    nc.sync.dma_start(
                        out=dg[:, :G],
                        in_=deg[row0 : row0 + nrow].rearrange("(p g) -> p g", p=128),
                    )
                    nc.vector.reciprocal(dg[:, :G], dg[:, :G])
                    nc.scalar.sqrt(dg[:, :G], dg[:, :G])
                    nc.vector.tensor_tensor(
                        out=ft[:, :G, :],
                        in0=ft[:, :G, :],
                        in1=dg[:, :G].unsqueeze(2).to_broadcast([128, G, F]),
                        op=mybir.AluOpType.mult,
                    )
                    lrow = row0 - c * CHUNK
                    nc.sync.dma_start(
                        out=ftabs[c][lrow : lrow + nrow, :].rearrange(
                            "(p g) f -> p g f", p=128
                        ),
                        in_=ft[:, :G, :],
                    )

            # --- phase 2: gather + PE segment-sum -----------------------
            # last chunk (in processing order) with groups for each window
            last_chunk = {}
            for c in CHUNK_ORDER:
                for w, ngrp in structure[c]:
                    last_chunk[w] = c
            gpg = TILE_E // 128    # groups per gather tile (8)
            with (
                tc.tile_pool(name="pre", bufs=3) as pp,
                tc.tile_pool(name="gath", bufs=gbufs) as gp,
                tc.tile_pool(name="sp", bufs=sbufs) as spool,
                tc.tile_pool(name="aps", bufs=psbufs, space="PSUM") as apsp,
                tc.tile_pool(name="fin", bufs=3) as fp,
                tc.tile_pool(name="fps", bufs=2, space="PSUM") as fpsp,
            ):
                done_win = set()

                def finalize(w):
                    # h[m,:] = s[m] * (agg[m,:] @ W) + b   (row-scale commutes
                    # through the right-matmul)
                    done_win.add(w)
                    tp = fpsp.tile([F, 128], F32, tag="tp")
                    nc.tensor.transpose(
                        out=tp[:, :], in_=agg_sb[:, w, :], identity=ident[:, :]
                    )
                    acc = fp.tile([F, 128], F32, tag="acc")
                    nc.scalar.activation(
                        acc[:, :], tp[:, :], mybir.ActivationFunctionType.Copy
                    )
                    ot = fpsp.tile([128, F], F32, tag="ot")
                    nc.tensor.matmul(
                        out=ot[:, :], lhsT=acc[:, :], rhs=wb[:, :],
                        start=True, stop=True,
                    )
                    nc.vector.scalar_tensor_tensor(
                        out=osb_all[:, w, :], in0=ot[:, :],
                        scalar=s_own[:, w : w + 1], in1=b_mat[:, :],
                        op0=mybir.AluOpType.mult, op1=mybir.AluOpType.add,
                    )
                gdict = {}   # global tile index -> gather tile
                sdict = {}   # tile-pair index -> S tile ([128, 2*gpg, 128])

                def get_tile(gti, c):
                    # tile gti covers tokens [gti*TILE_E, (gti+1)*TILE_E)
                    if gti not in gdict:
                        icol = gti * TILE_E // 16
                        gt = gp.tile([128, gpg, F], F32, tag="gt")
                        if ab_gather:
                            nc.gpsimd.dma_gather(
                                gt[:, :, :],
                                ftabs[c][:, :],
                                gidx_sb[:, icol : icol + TILE_E // 16],
                                TILE_E,
                                TILE_E,
                                F,
                                queue_num=gti % nqueues,
                            )
                        else:
                            nc.vector.memset(gt[:, 0:1, 0:1], 0.0)
                        gdict[gti] = gt
                    if gti not in sdict:
                        st = spool.tile([128, gpg, 128], F32, tag="st")
                        dcol = gti * gpg
                        if ab_sbuild:
                            nc.vector.tensor_tensor(
                                out=st[:, :, :],
                                in0=drel_sb[:, dcol : dcol + gpg]
                                .unsqueeze(2)
                                .to_broadcast([128, gpg, 128]),
                                in1=iota_mat[:, :].unsqueeze(1).to_broadcast(
                                    [128, gpg, 128]),
                                op=mybir.AluOpType.is_equal,
                            )
                        else:
                            nc.vector.memset(st[:, 0:1, 0:1], 0.0)
                        sdict[gti] = st
                    return gdict[gti], sdict[gti]

                base_tok = 0
                for c in CHUNK_ORDER:
                    prescale(pp, c)
                    g_cursor = base_tok // 128   # global group index
                    # quads of consecutive windows share one psum tile so a
                    # single DVE op flushes all of them (DVE per-op overhead
                    # dominates the tiny [128,64] adds)
                    ents = structure[c]
                    i = 0
                    while i < len(ents):
                        quad = [ents[i]]
                        while (len(quad) < 4 and i + len(quad) < len(ents)
                               and ents[i + len(quad)][0] == quad[-1][0] + 1):
                            quad.append(ents[i + len(quad)])
                        q = len(quad)
                        w0 = quad[0][0]
                        ps = apsp.tile([128, 8, F], F32, tag="ps")
                        for j, (w, ngrp) in enumerate(quad):
                            if not ab_matmul:
                                nc.vector.memset(ps[:, j, :], 0.0)
                                g_cursor += ngrp
                                continue
                            for gi in range(ngrp):
                                gcol = g_cursor + gi
                                gti = gcol // gpg
                                gt, st = get_tile(gti, c)
                                nc.tensor.matmul(
                                    out=ps[:, j, :],
                                    lhsT=st[:, gcol % gpg, :],
                                    rhs=gt[:, gcol % gpg, :],
                                    start=(gi == 0),
                                    stop=(gi == ngrp - 1),
                                )
                            g_cursor += ngrp
                        nc.vector.tensor_tensor(
                            out=agg_sb[:, w0 : w0 + q, :],
                            in0=agg_sb[:, w0 : w0 + q, :],
                            in1=ps[:, 0:q, :],
                            op=mybir.AluOpType.add,
                        )
                        if ab_final:
                            for w, _ in quad:
                                if last_chunk[w] == c:
                                    finalize(w)
                        i += q
                    base_tok += caps[c]

                # windows with no edges at all: bias-only output
                if ab_final:
                    for w in range(N_WIN):
                        if w not in done_win:
                            finalize(w)
                    # batched output writes: windows 0..72 full + 73 partial
                    nc.sync.dma_start(
                        out=out[0 : 73 * 128, :].rearrange(
                            "(w p) f -> p w f", p=128
                        ),
                        in_=osb_all[:, 0:73, :],
                    )
                    nc.sync.dma_start(
                        out=out[73 * 128 : OWN, :],
                        in_=osb_all[0 : OWN - 73 * 128, 73, :],
                    )
    nc.compile()
    return nc


def _pack_idx(vals):
    """Token i -> [i%16 + 16c, i//16] for c in 0..7 (wrap-16, replicated)."""
    cols = len(vals) // 16
    a = vals.reshape(cols, 16).T
    return np.tile(a, (8, 1)).astype(np.int16)


def _pack_tok128(vals):
    """Token i -> [i%128, i//128] layout, int16."""
    cols = len(vals) // 128
    return np.ascontiguousarray(vals.reshape(cols, 128).T.astype(np.int16))


def _prepare(feature, degree, src, dst, W, b):
    src = np.asarray(src).astype(np.int64)
    dst = np.asarray(dst).astype(np.int64)
    core = dst // OWN
    chunk = src // CHUNK
    ldst = dst - core * OWN
    win = ldst // 128

    # counts[k, c, w]
    key = (core * 3 + chunk) * N_WIN + win
    counts = np.bincount(key, minlength=NCORES * 3 * N_WIN).reshape(
        NCORES, 3, N_WIN
    )
    G = np.maximum.reduce(-(-counts // 128), axis=0)  # [3, N_WIN] group counts

    # shared program structure + per-chunk caps
    structure = []
    caps = []
    for c in range(3):
        st = [(w, int(G[c, w])) for w in range(N_WIN) if G[c, w] > 0]
        structure.append(st)
        ntok = int(G[c].sum()) * 128
        caps.append(max(TILE_E, -(-ntok // TILE_E) * TILE_E))

    # slot offsets of each (c, w) segment; chunks laid out in CHUNK_ORDER
    base = 0
    seg_off = np.zeros((3, N_WIN), np.int64)
    for c in CHUNK_ORDER:
        off = base
        for w in range(N_WIN):
            seg_off[c, w] = off
            off += int(G[c, w]) * 128
        base += caps[c]
    tot = int(base)

    feat_pad = np.zeros((PAD_N, F), np.float32)
    feat_pad[:N_NODES] = np.asarray(feature, np.float32)
    deg_pad = np.ones(PAD_N, np.float32)
    deg_pad[:N_NODES] = np.asarray(degree, np.float32)
    Wn = np.ascontiguousarray(np.asarray(W, np.float32))
    bn = np.ascontiguousarray(np.asarray(b, np.float32))
    degree_np = np.asarray(degree, np.float32)

    # per-core token placement: edges of (k,c,w) go to consecutive slots at
    # seg_off[c,w]
    order = np.argsort(key, kind="stable")
    skey = key[order]
    kstarts = np.concatenate([[0], np.cumsum(np.bincount(
        skey, minlength=NCORES * 3 * N_WIN))])

    in_maps = []
    for k in range(NCORES):
        gv = np.zeros(tot, np.int64)
        dv = np.full(tot, -1, np.int64)
        for c in range(3):
            for w in range(N_WIN):
                if G[c, w] == 0:
                    continue
                b0 = kstarts[(k * 3 + c) * N_WIN + w]
                b1 = kstarts[(k * 3 + c) * N_WIN + w + 1]
                n = b1 - b0
                o = seg_off[c, w]
                e = order[b0:b1]
                gv[o : o + n] = src[e] - c * CHUNK
                dv[o : o + n] = ldst[e] - w * 128
        do = np.ones(AGG_ROWS, np.float32)
        do[:OWN] = degree_np[k * OWN : (k + 1) * OWN]
        in_maps.append(
            {
                "feature": feat_pad,
                "degree": deg_pad,
                "gidx": _pack_idx(gv),
                "drel": _pack_tok128(dv),
                "deg_own": do,
                "W": Wn,
                "b": bn,
            }
        )
    return structure, tuple(caps), in_maps


def kernel(feature, degree, src, dst, W, b):
    structure, caps, in_maps = _prepare(feature, degree, src, dst, W, b)
    nc = _build_nc(structure, caps)
    res = run_bass_kernel_spmd(nc, in_maps, list(range(NCORES)))
    outp = np.concatenate([res.results[k]["out"] for k in range(NCORES)], axis=0)
    return outp.astype(np.float32)
